# revision 25
# baseline (speedup 1.0000x reference)
"""Trainium2 Bass kernel for nn_DKL_45810121179236 (retrieval_knn).

Reference computation:
    C = cos_sim_matrix(ex, ey)            # [8192, 8192], D=256
    out1 = -sum(exp(c1)*c1), c1 = logN(1 - rowmax(C))
    out2 = -sum(exp(c2)*c2), c2 = logN(1 - colmax(C))

Strategy: the ACT drain of each PSUM group applies exp(beta*(C - mhat))
(one pass it must make anyway), which gives BOTH outputs nearly free:
  rowmax_i = mhat + ln(sum_j exp(beta*(C_ij-mhat)))/beta - debias
      via the ACTIVATE accum_out per-row sum (logsumexp == max for large
      beta; the EVT first-order bias ln(1+1/(beta*theta))/beta is
      subtracted on host),
  colmax_j = mhat + ln(max_i cf_ij)/beta
      via the DVE running tensor_max over the monotone exp-space tiles.
This deletes the per-tile DVE rowmax trees (~40us) from the old design.
beta and -beta*mhat ride in as a [128,2] runtime tensor (per-partition
scale/bias APs), so the NEFF never rebuilds; the host picks beta from a
subsampled estimate of the rowmax/colmax range to keep all exponents in
f32/bf16 range.

Device per core (ex rows sharded 8 ways):
  mm:    per x-tile (128 rows), 4 PSUM groups [128, 2048] f32,
         2 K-chunks x 4 N=512 bf16 matmuls per group (PE)
  drain: ACT Exp(scale*C+bias) PSUM->SBUF bf16 + accum_out row-sum
  colmax: running elementwise max into colacc [128, 8192] (DVE TT 2x bf16)
  out:   rsums [128, 32] f32 (per x-tile per group), colacc bf16 -> host
"""

import sys

sys.path.insert(0, "/opt/trn_rl_repo")

import copy
from contextlib import ExitStack

import numpy as np
import ml_dtypes

import concourse.bass as bass
import concourse.tile as tile
from concourse import mybir
from concourse import bass_utils

# NOTE: walrus --enable-ldw-opt=true was tried to dedup per-matmul
# LDWEIGHTS; it crashes codegen (visitInstLdweights INTERNAL_ERROR), so
# the per-MM weight reloads stay.

N_CORES = 8
N = 8192  # rows in ex and ey
D = 256  # embedding dim
XR = N // N_CORES  # ex rows per core (1024)
NT_X = XR // 128  # 8 x-tiles per core
PSW = 2048  # psum group width (4 banks)
NG = N // PSW  # 4 psum groups per x-tile
PLW = 1536  # plain-C tail stripe width drained by DVE (cols N-PLW..N)

F32 = mybir.dt.float32
BF16 = mybir.dt.bfloat16
AF = mybir.ActivationFunctionType
ALU = mybir.AluOpType
AX = mybir.AxisListType

SIGMA = 0.3
EPS = 1e-8
ZPOS = 86.0  # exp budget above mhat (bf16 overflow at e^88.7)
ZNEG = 88.0  # exp budget below mhat (f32/bf16 underflow ~ e^-87)
BETA_MAX = 1500.0


def _split_multi_waits(nc, max_waits=1):
    """The walrus build in this container rejects instructions carrying more
    than one sync wait. Move excess waits onto preceding same-engine NOPs
    (waits on one engine are sequential, so semantics are unchanged)."""
    n_split = 0
    for function in nc.m.functions:
        new_blocks = []
        for block in function.blocks:
            new_insts = []
            for inst in block.instructions:
                si = inst.sync_info
                if si is not None and si.on_wait and len(si.on_wait) > max_waits:
                    waits = list(si.on_wait)
                    n_split += 1
                    head, rest = waits[:-max_waits], waits[-max_waits:]
                    for ci in range(0, len(head), max_waits):
                        new_insts.append(
                            mybir.InstNoOp(
                                name=f"{inst.name}-ws{ci}",
                                engine=inst.engine,
                                sync_info=mybir.SyncInfo(
                                    on_wait=head[ci : ci + max_waits], on_update=[]
                                ),
                            )
                        )
                    inst = copy.replace(
                        inst,
                        sync_info=mybir.SyncInfo(
                            on_wait=rest, on_update=list(si.on_update)
                        ),
                    )
                new_insts.append(inst)
            new_blocks.append(copy.replace(block, instructions=new_insts))
        function.blocks.clear()
        for b in new_blocks:
            function.blocks.append(b)
    return n_split


def _build():
    nc = bass.Bass("TRN2", target_bir_lowering=False, debug=False, num_devices=1)
    # host packs x as [p, h*XR+c] = xn.T[h*128+p, c] (both K-halves along
    # free axis, contiguous 4KB rows); y as [2g+h, p, c] chunk-major blocks
    xT = nc.dram_tensor("xT", [128, 2 * XR], BF16, kind="ExternalInput").ap()
    yT = nc.dram_tensor("yT", [2 * NG, 128, PSW], BF16, kind="ExternalInput").ap()
    sb = nc.dram_tensor("sb", [128, 2], F32, kind="ExternalInput").ap()
    rs_o = nc.dram_tensor("rsums", [128, NT_X * NG], F32, kind="ExternalOutput").ap()
    rowp_o = nc.dram_tensor("rowp", [128, NT_X], F32, kind="ExternalOutput").ap()
    colp_o = nc.dram_tensor("colp", [128, N], BF16, kind="ExternalOutput").ap()

    with tile.TileContext(nc) as tc:
        with ExitStack() as ctx:
            ep = ctx.enter_context

            persist = ep(tc.tile_pool(name="persist", bufs=1))
            xTab = persist.tile([128, 2 * XR], BF16, tag="xTab")
            yTa = persist.tile([128, N], BF16, tag="yTa")
            yTb = persist.tile([128, N], BF16, tag="yTb")
            colacc = persist.tile([128, N], BF16, tag="colacc")
            sbt = persist.tile([128, 2], F32, tag="sbt")
            rsums = persist.tile([128, NT_X * NG], F32, tag="rsums")
            rowp = persist.tile([128, NT_X], F32, tag="rowp")
            rt1 = persist.tile([128, PLW // 2], BF16, tag="rt1")
            rt2 = persist.tile([128, PLW // 4], BF16, tag="rt2")

            # input DMAs spread over 4 engine HWDGE queues (contiguous
            # sources); ordered so mm group 0's tiles land first
            def ya(g):
                return (yTa[:, g * PSW : (g + 1) * PSW], yT[2 * g])

            def yb(g):
                return (yTb[:, g * PSW : (g + 1) * PSW], yT[2 * g + 1])

            # first chunks split in half for a shorter pipeline head
            nc.sync.dma_start(yTa[:, 0:1024], yT[0, :, 0:1024])
            nc.sync.dma_start(yTa[:, 1024:2048], yT[0, :, 1024:2048])
            nc.sync.dma_start(*ya(1))
            nc.sync.dma_start(*ya(3))
            nc.scalar.dma_start(yTb[:, 0:1024], yT[1, :, 0:1024])
            nc.scalar.dma_start(yTb[:, 1024:2048], yT[1, :, 1024:2048])
            nc.scalar.dma_start(*yb(1))
            nc.scalar.dma_start(*yb(3))
            nc.gpsimd.dma_start(xTab[:, 0:XR], xT[:, 0:XR])
            nc.gpsimd.dma_start(xTab[:, XR : 2 * XR], xT[:, XR : 2 * XR])
            nc.gpsimd.dma_start(sbt[:], sb)
            nc.gpsimd.dma_start(*ya(2))
            nc.gpsimd.dma_start(*yb(2))

            mm_pool = ep(tc.tile_pool(name="mm", bufs=2, space="PSUM"))
            cf_pool = ep(tc.tile_pool(name="cf", bufs=2))

            for xt in range(NT_X):
                xa = slice(xt * 128, (xt + 1) * 128)
                xb = slice(XR + xt * 128, XR + (xt + 1) * 128)
                cf = cf_pool.tile([128, N], BF16, tag="cf")
                # group-pair interleaved matmuls: [g.a g+1.a g.b g+1.b]
                # halves the PE weight switches (same xT slice across groups)
                for gp in range(0, NG, 2):
                    pss = [
                        mm_pool.tile([128, PSW], F32, tag="mm", name=f"mm{xt}_{gp}_{gi}")
                        for gi in range(2)
                    ]
                    for hx, ys in ((xa, yTa), (xb, yTb)):
                        for gi in range(2):
                            j0 = (gp + gi) * PSW
                            for c in range(PSW // 512):
                                nc.tensor.matmul(
                                    pss[gi][:, c * 512 : (c + 1) * 512],
                                    xTab[:, hx],
                                    ys[:, j0 + c * 512 : j0 + (c + 1) * 512],
                                    start=hx == xa,
                                    stop=hx == xb,
                                )
                    for gi in range(2):
                        g = gp + gi
                        j0 = g * PSW
                        ps = pss[gi]
                        if g < NG - 1:
                            # fused drain: cf = exp(beta*C - beta*mhat) bf16,
                            # rsums[:, xt*NG+g] = sum_j cf (f32)
                            nc.scalar.activation(
                                cf[:, j0 : j0 + PSW],
                                ps[:],
                                AF.Exp,
                                bias=sbt[:, 1:2],
                                scale=sbt[:, 0:1],
                                accum_out=rsums[:, xt * NG + g : xt * NG + g + 1],
                            )
                        else:
                            # drain-split: ACT exp-drains the head of the
                            # last group; DVE copies the plain-C tail stripe
                            # and row-reduces it (rebalances ACT vs DVE)
                            nc.scalar.activation(
                                cf[:, j0 : j0 + (PSW - PLW)],
                                ps[:, 0 : PSW - PLW],
                                AF.Exp,
                                bias=sbt[:, 1:2],
                                scale=sbt[:, 0:1],
                                accum_out=rsums[:, xt * NG + g : xt * NG + g + 1],
                            )
                            nc.vector.tensor_copy(
                                cf[:, N - PLW : N], ps[:, PSW - PLW : PSW]
                            )
                            nc.vector.tensor_max(
                                rt1[:],
                                cf[:, N - PLW : N - PLW // 2],
                                cf[:, N - PLW // 2 : N],
                            )
                            nc.vector.tensor_max(
                                rt2[:], rt1[:, 0 : PLW // 4], rt1[:, PLW // 4 :]
                            )
                            nc.vector.reduce_max(
                                rowp[:, xt : xt + 1], rt2[:], axis=AX.X
                            )

                # colmax chain in exp space (DVE TT 2x / 4x-copy init);
                # last tile chunked so the 2MB writeback overlaps the TTs
                if xt == 0:
                    nc.vector.tensor_copy(colacc[:], cf[:])
                elif xt < NT_X - 1:
                    nc.vector.tensor_max(colacc[:], colacc[:], cf[:])
                else:
                    for j0 in range(0, N, PSW):
                        je = j0 + PSW
                        if j0 < N - PSW:
                            nc.vector.tensor_max(
                                colacc[:, j0:je], colacc[:, j0:je], cf[:, j0:je]
                            )
                            nc.sync.dma_start(colp_o[:, j0:je], colacc[:, j0:je])
                        else:
                            # finest split on the very last chunk to cut the
                            # writeback tail
                            h = PSW // 2
                            nc.vector.tensor_max(
                                colacc[:, j0 : j0 + h],
                                colacc[:, j0 : j0 + h],
                                cf[:, j0 : j0 + h],
                            )
                            nc.sync.dma_start(
                                colp_o[:, j0 : j0 + h], colacc[:, j0 : j0 + h]
                            )
                            nc.vector.tensor_max(
                                colacc[:, j0 + h : je],
                                colacc[:, j0 + h : je],
                                cf[:, j0 + h : je],
                            )
                            nc.sync.dma_start(
                                colp_o[:, j0 + h : je], colacc[:, j0 + h : je]
                            )

            nc.scalar.dma_start(rs_o, rsums[:])
            nc.scalar.dma_start(rowp_o, rowp[:])

    _split_multi_waits(nc)
    return nc


_NC_CACHE = []


def _get_nc():
    if not _NC_CACHE:
        _NC_CACHE.append(_build())
    return _NC_CACHE[0]


def run_device(ex, ey, trace=False):
    """Normalize/transposed-shard on host, run SPMD kernel, return
    (rowmax [N], colmax [N], results obj)."""
    nc = _get_nc()
    xn = ex / np.maximum(np.linalg.norm(ex, axis=-1, keepdims=True), EPS)
    yn = ey / np.maximum(np.linalg.norm(ey, axis=-1, keepdims=True), EPS)
    xTf = np.ascontiguousarray(xn.T).astype(ml_dtypes.bfloat16)  # [256, 8192]
    yTf = np.ascontiguousarray(yn.T).astype(ml_dtypes.bfloat16)
    # chunk-major contiguous device layouts (see _build)
    yTq = np.ascontiguousarray(
        yTf.reshape(2, 128, NG, PSW).transpose(2, 0, 1, 3).reshape(2 * NG, 128, PSW)
    )

    # host-side beta/mhat selection from a subsampled range estimate
    S = 4
    rm_est = (xn @ yn[::S].T).max(axis=1)
    cm_est = (xn[::S] @ yn.T).max(axis=0)
    sigma_c = float(np.std((xn[::97] @ yn[::89].T)))
    z_full = np.sqrt(2.0 * np.log(float(N)))
    z_sub = np.sqrt(2.0 * np.log(float(N) / S))
    delta = sigma_c * (z_full - z_sub)
    hi = float(max(rm_est.max(), cm_est.max())) + 0.65 * delta + 0.004
    lo = float(min(rm_est.min(), cm_est.min())) + 0.55 * delta - 0.004
    hi = min(hi, 1.0)  # cosine bound
    lo = min(lo, hi - 1e-3)
    beta = min(BETA_MAX, (ZPOS + ZNEG) / (hi - lo))
    mhat = hi - ZPOS / beta
    theta = sigma_c / z_full
    debias = float(np.log1p(1.0 / (beta * theta)) / beta)

    sbv = np.empty((128, 2), dtype=np.float32)
    sbv[:, 0] = beta
    sbv[:, 1] = -beta * mhat

    in_maps = [
        {
            "xT": np.ascontiguousarray(
                xTf[:, k * XR : (k + 1) * XR]
                .reshape(2, 128, XR)
                .transpose(1, 0, 2)
                .reshape(128, 2 * XR)
            ),
            "yT": yTq,
            "sb": sbv,
        }
        for k in range(N_CORES)
    ]
    res = bass_utils.run_bass_kernel_spmd(
        nc, in_maps, core_ids=list(range(N_CORES)), trace=trace
    )
    rowmax = np.empty(N, dtype=np.float64)
    colps = []
    for k in range(N_CORES):
        rs = np.asarray(res.results[k]["rsums"], dtype=np.float64)  # [128, 32]
        # row xt*128+p of this shard -> rsums[p, xt*NG:(xt+1)*NG]
        rs = rs.reshape(128, NT_X, NG).sum(axis=2)  # [128, NT_X]
        rowsum = np.maximum(rs.T.reshape(-1), 1e-300)  # [XR], row-major
        rm_exp = mhat + np.log(rowsum) / beta - debias
        # plain-C tail stripe rowmax partial
        rp = np.asarray(res.results[k]["rowp"], dtype=np.float64)  # [128, NT_X]
        rowmax[k * XR : (k + 1) * XR] = np.maximum(rm_exp, rp.T.reshape(-1))
        colps.append(np.asarray(res.results[k]["colp"]).astype(np.float32))
    colraw = np.max(np.stack(colps), axis=(0, 1)).astype(np.float64)  # [N]
    colmax = np.empty(N, dtype=np.float64)
    colmax[: N - PLW] = mhat + np.log(np.maximum(colraw[: N - PLW], 1e-300)) / beta
    colmax[N - PLW :] = colraw[N - PLW :]  # plain-C stripe
    return rowmax, colmax, res


def _entropy(m):
    # -sum(exp(c)*c), c = logprob_Normal(1,SIGMA)(1 - m); accumulate in f64
    z = -m.astype(np.float64) / SIGMA
    c = -0.5 * z * z - np.log(SIGMA) - 0.5 * np.log(2.0 * np.pi)
    return -np.sum(np.exp(c) * c)


def kernel(ex, ey):
    ex = np.ascontiguousarray(np.asarray(ex), dtype=np.float32)
    ey = np.ascontiguousarray(np.asarray(ey), dtype=np.float32)
    rowmax, colmax, _ = run_device(ex, ey)
    out1 = np.float32(_entropy(rowmax))
    out2 = np.float32(_entropy(colmax))
    return (np.asarray(out1, dtype=np.float32), np.asarray(out2, dtype=np.float32))


# revision 29
# speedup vs baseline: 1.0066x; 1.0066x over previous
"""Trainium2 Bass kernel for nn_DKL_45810121179236 (retrieval_knn).

Reference computation:
    C = cos_sim_matrix(ex, ey)            # [8192, 8192], D=256
    out1 = -sum(exp(c1)*c1), c1 = logN(1 - rowmax(C))
    out2 = -sum(exp(c2)*c2), c2 = logN(1 - colmax(C))

Strategy: the ACT drain of each PSUM group applies exp(beta*(C - mhat))
(one pass it must make anyway), which gives BOTH outputs nearly free:
  rowmax_i = mhat + ln(sum_j exp(beta*(C_ij-mhat)))/beta - debias
      via the ACTIVATE accum_out per-row sum (logsumexp == max for large
      beta; the EVT first-order bias ln(1+1/(beta*theta))/beta is
      subtracted on host),
  colmax_j = mhat + ln(max_i cf_ij)/beta
      via the DVE running tensor_max over the monotone exp-space tiles.
This deletes the per-tile DVE rowmax trees (~40us) from the old design.
beta and -beta*mhat ride in as a [128,2] runtime tensor (per-partition
scale/bias APs), so the NEFF never rebuilds; the host picks beta from a
subsampled estimate of the rowmax/colmax range to keep all exponents in
f32/bf16 range.

Device per core (ex rows sharded 8 ways):
  mm:    per x-tile (128 rows), 4 PSUM groups [128, 2048] f32,
         2 K-chunks x 4 N=512 bf16 matmuls per group (PE)
  drain: ACT Exp(scale*C+bias) PSUM->SBUF bf16 + accum_out row-sum
  colmax: running elementwise max into colacc [128, 8192] (DVE TT 2x bf16)
  out:   rsums [128, 32] f32 (per x-tile per group), colacc bf16 -> host
"""

import sys

sys.path.insert(0, "/opt/trn_rl_repo")

import copy
from contextlib import ExitStack

import numpy as np
import ml_dtypes

import concourse.bass as bass
import concourse.tile as tile
from concourse import mybir
from concourse import bass_utils

# NOTE: walrus --enable-ldw-opt=true was tried to dedup per-matmul
# LDWEIGHTS; it crashes codegen (visitInstLdweights INTERNAL_ERROR), so
# the per-MM weight reloads stay.

N_CORES = 8
N = 8192  # rows in ex and ey
D = 256  # embedding dim
XR = N // N_CORES  # ex rows per core (1024)
NT_X = XR // 128  # 8 x-tiles per core
PSW = 2048  # psum group width (4 banks)
NG = N // PSW  # 4 psum groups per x-tile
PLW = 1408  # plain-C tail stripe width drained by DVE (cols N-PLW..N)
MMW = 512  # matmul moving-operand width (1024 bf16 rejected by walrus codegen)

F32 = mybir.dt.float32
BF16 = mybir.dt.bfloat16
AF = mybir.ActivationFunctionType
ALU = mybir.AluOpType
AX = mybir.AxisListType

SIGMA = 0.3
EPS = 1e-8
ZPOS = 86.0  # exp budget above mhat (bf16 overflow at e^88.7)
ZNEG = 88.0  # exp budget below mhat (f32/bf16 underflow ~ e^-87)
BETA_MAX = 1500.0


def _split_multi_waits(nc, max_waits=1):
    """The walrus build in this container rejects instructions carrying more
    than one sync wait. Move excess waits onto preceding same-engine NOPs
    (waits on one engine are sequential, so semantics are unchanged)."""
    n_split = 0
    for function in nc.m.functions:
        new_blocks = []
        for block in function.blocks:
            new_insts = []
            for inst in block.instructions:
                si = inst.sync_info
                if si is not None and si.on_wait and len(si.on_wait) > max_waits:
                    waits = list(si.on_wait)
                    n_split += 1
                    head, rest = waits[:-max_waits], waits[-max_waits:]
                    for ci in range(0, len(head), max_waits):
                        new_insts.append(
                            mybir.InstNoOp(
                                name=f"{inst.name}-ws{ci}",
                                engine=inst.engine,
                                sync_info=mybir.SyncInfo(
                                    on_wait=head[ci : ci + max_waits], on_update=[]
                                ),
                            )
                        )
                    inst = copy.replace(
                        inst,
                        sync_info=mybir.SyncInfo(
                            on_wait=rest, on_update=list(si.on_update)
                        ),
                    )
                new_insts.append(inst)
            new_blocks.append(copy.replace(block, instructions=new_insts))
        function.blocks.clear()
        for b in new_blocks:
            function.blocks.append(b)
    return n_split


def _build():
    nc = bass.Bass("TRN2", target_bir_lowering=False, debug=False, num_devices=1)
    # host packs x as [p, h*XR+c] = xn.T[h*128+p, c] (both K-halves along
    # free axis, contiguous 4KB rows); y as [2g+h, p, c] chunk-major blocks
    xT = nc.dram_tensor("xT", [128, 2 * XR], BF16, kind="ExternalInput").ap()
    yT = nc.dram_tensor("yT", [2 * NG, 128, PSW], BF16, kind="ExternalInput").ap()
    sb = nc.dram_tensor("sb", [128, 2], F32, kind="ExternalInput").ap()
    rs_o = nc.dram_tensor("rsums", [128, NT_X * NG], F32, kind="ExternalOutput").ap()
    rowp_o = nc.dram_tensor("rowp", [128, NT_X], F32, kind="ExternalOutput").ap()
    colp_o = nc.dram_tensor("colp", [128, N], BF16, kind="ExternalOutput").ap()

    with tile.TileContext(nc) as tc:
        with ExitStack() as ctx:
            ep = ctx.enter_context

            persist = ep(tc.tile_pool(name="persist", bufs=1))
            xTab = persist.tile([128, 2 * XR], BF16, tag="xTab")
            yTa = persist.tile([128, N], BF16, tag="yTa")
            yTb = persist.tile([128, N], BF16, tag="yTb")
            colacc = persist.tile([128, N], BF16, tag="colacc")
            sbt = persist.tile([128, 2], F32, tag="sbt")
            rsums = persist.tile([128, NT_X * NG], F32, tag="rsums")
            rowp = persist.tile([128, NT_X], F32, tag="rowp")
            rt1 = persist.tile([128, PLW // 2], BF16, tag="rt1")
            rt2 = persist.tile([128, PLW // 4], BF16, tag="rt2")

            # input DMAs spread over 4 engine HWDGE queues (contiguous
            # sources); ordered so mm group 0's tiles land first
            def ya(g):
                return (yTa[:, g * PSW : (g + 1) * PSW], yT[2 * g])

            def yb(g):
                return (yTb[:, g * PSW : (g + 1) * PSW], yT[2 * g + 1])

            # first chunks split in half for a shorter pipeline head
            nc.sync.dma_start(yTa[:, 0:1024], yT[0, :, 0:1024])
            nc.sync.dma_start(yTa[:, 1024:2048], yT[0, :, 1024:2048])
            nc.sync.dma_start(*ya(1))
            nc.sync.dma_start(*ya(3))
            nc.scalar.dma_start(yTb[:, 0:1024], yT[1, :, 0:1024])
            nc.scalar.dma_start(yTb[:, 1024:2048], yT[1, :, 1024:2048])
            nc.scalar.dma_start(*yb(1))
            nc.scalar.dma_start(*yb(3))
            nc.gpsimd.dma_start(xTab[:, 0:XR], xT[:, 0:XR])
            nc.gpsimd.dma_start(xTab[:, XR : 2 * XR], xT[:, XR : 2 * XR])
            nc.gpsimd.dma_start(sbt[:], sb)
            nc.gpsimd.dma_start(*ya(2))
            nc.gpsimd.dma_start(*yb(2))

            mm_pool = ep(tc.tile_pool(name="mm", bufs=2, space="PSUM"))
            cf_pool = ep(tc.tile_pool(name="cf", bufs=2))

            for xt in range(NT_X):
                xa = slice(xt * 128, (xt + 1) * 128)
                xb = slice(XR + xt * 128, XR + (xt + 1) * 128)
                cf = cf_pool.tile([128, N], BF16, tag="cf")
                for g in range(NG):
                    j0 = g * PSW
                    ps = mm_pool.tile([128, PSW], F32, tag="mm")
                    for c in range(PSW // MMW):
                        nc.tensor.matmul(
                            ps[:, c * MMW : (c + 1) * MMW],
                            xTab[:, xa],
                            yTa[:, j0 + c * MMW : j0 + (c + 1) * MMW],
                            start=True,
                            stop=False,
                        )
                    for c in range(PSW // MMW):
                        nc.tensor.matmul(
                            ps[:, c * MMW : (c + 1) * MMW],
                            xTab[:, xb],
                            yTb[:, j0 + c * MMW : j0 + (c + 1) * MMW],
                            start=False,
                            stop=True,
                        )
                    if g < NG - 1:
                        # fused drain: cf = exp(beta*C - beta*mhat) bf16,
                        # rsums[:, xt*NG+g] = sum_j cf (f32)
                        nc.scalar.activation(
                            cf[:, j0 : j0 + PSW],
                            ps[:],
                            AF.Exp,
                            bias=sbt[:, 1:2],
                            scale=sbt[:, 0:1],
                            accum_out=rsums[:, xt * NG + g : xt * NG + g + 1],
                        )
                    else:
                        # drain-split: ACT exp-drains the head of the last
                        # group; DVE copies the plain-C tail stripe and
                        # row-reduces it (rebalances ACT vs DVE)
                        nc.scalar.activation(
                            cf[:, j0 : j0 + (PSW - PLW)],
                            ps[:, 0 : PSW - PLW],
                            AF.Exp,
                            bias=sbt[:, 1:2],
                            scale=sbt[:, 0:1],
                            accum_out=rsums[:, xt * NG + g : xt * NG + g + 1],
                        )
                        nc.vector.tensor_copy(
                            cf[:, N - PLW : N], ps[:, PSW - PLW : PSW]
                        )

                    # colmax chunk for (xt, g) right after its drain: keeps
                    # the DVE queue fine-grained (PSUM recycle never waits a
                    # 4.3us full-width TT) and shrinks the end tail
                    if xt == 0:
                        nc.vector.tensor_copy(
                            colacc[:, j0 : j0 + PSW], cf[:, j0 : j0 + PSW]
                        )
                    else:
                        nc.vector.tensor_max(
                            colacc[:, j0 : j0 + PSW],
                            colacc[:, j0 : j0 + PSW],
                            cf[:, j0 : j0 + PSW],
                        )
                        if xt == NT_X - 1:
                            nc.sync.dma_start(
                                colp_o[:, j0 : j0 + PSW], colacc[:, j0 : j0 + PSW]
                            )
                    if g == NG - 1:
                        # plain-stripe rowmax tree (after the colmax chunk so
                        # the last tile's writeback isn't delayed behind it)
                        nc.vector.tensor_max(
                            rt1[:],
                            cf[:, N - PLW : N - PLW // 2],
                            cf[:, N - PLW // 2 : N],
                        )
                        nc.vector.tensor_max(
                            rt2[:], rt1[:, 0 : PLW // 4], rt1[:, PLW // 4 :]
                        )
                        nc.vector.reduce_max(
                            rowp[:, xt : xt + 1], rt2[:], axis=AX.X
                        )

            nc.scalar.dma_start(rs_o, rsums[:])
            nc.scalar.dma_start(rowp_o, rowp[:])

    _split_multi_waits(nc)
    return nc


_NC_CACHE = []


def _get_nc():
    if not _NC_CACHE:
        _NC_CACHE.append(_build())
    return _NC_CACHE[0]


def run_device(ex, ey, trace=False):
    """Normalize/transposed-shard on host, run SPMD kernel, return
    (rowmax [N], colmax [N], results obj)."""
    nc = _get_nc()
    xn = ex / np.maximum(np.linalg.norm(ex, axis=-1, keepdims=True), EPS)
    yn = ey / np.maximum(np.linalg.norm(ey, axis=-1, keepdims=True), EPS)
    xTf = np.ascontiguousarray(xn.T).astype(ml_dtypes.bfloat16)  # [256, 8192]
    yTf = np.ascontiguousarray(yn.T).astype(ml_dtypes.bfloat16)
    # chunk-major contiguous device layouts (see _build)
    yTq = np.ascontiguousarray(
        yTf.reshape(2, 128, NG, PSW).transpose(2, 0, 1, 3).reshape(2 * NG, 128, PSW)
    )

    # host-side beta/mhat selection from a subsampled range estimate
    S = 4
    rm_est = (xn @ yn[::S].T).max(axis=1)
    cm_est = (xn[::S] @ yn.T).max(axis=0)
    sigma_c = float(np.std((xn[::97] @ yn[::89].T)))
    z_full = np.sqrt(2.0 * np.log(float(N)))
    z_sub = np.sqrt(2.0 * np.log(float(N) / S))
    delta = sigma_c * (z_full - z_sub)
    hi = float(max(rm_est.max(), cm_est.max())) + 0.65 * delta + 0.004
    lo = float(min(rm_est.min(), cm_est.min())) + 0.55 * delta - 0.004
    hi = min(hi, 1.0)  # cosine bound
    lo = min(lo, hi - 1e-3)
    beta = min(BETA_MAX, (ZPOS + ZNEG) / (hi - lo))
    mhat = hi - ZPOS / beta
    theta = sigma_c / z_full
    debias = float(np.log1p(1.0 / (beta * theta)) / beta)

    sbv = np.empty((128, 2), dtype=np.float32)
    sbv[:, 0] = beta
    sbv[:, 1] = -beta * mhat

    in_maps = [
        {
            "xT": np.ascontiguousarray(
                xTf[:, k * XR : (k + 1) * XR]
                .reshape(2, 128, XR)
                .transpose(1, 0, 2)
                .reshape(128, 2 * XR)
            ),
            "yT": yTq,
            "sb": sbv,
        }
        for k in range(N_CORES)
    ]
    res = bass_utils.run_bass_kernel_spmd(
        nc, in_maps, core_ids=list(range(N_CORES)), trace=trace
    )
    rowmax = np.empty(N, dtype=np.float64)
    colps = []
    for k in range(N_CORES):
        rs = np.asarray(res.results[k]["rsums"], dtype=np.float64)  # [128, 32]
        # row xt*128+p of this shard -> rsums[p, xt*NG:(xt+1)*NG]
        rs = rs.reshape(128, NT_X, NG).sum(axis=2)  # [128, NT_X]
        rowsum = np.maximum(rs.T.reshape(-1), 1e-300)  # [XR], row-major
        rm_exp = mhat + np.log(rowsum) / beta - debias
        # plain-C tail stripe rowmax partial
        rp = np.asarray(res.results[k]["rowp"], dtype=np.float64)  # [128, NT_X]
        rowmax[k * XR : (k + 1) * XR] = np.maximum(rm_exp, rp.T.reshape(-1))
        colps.append(np.asarray(res.results[k]["colp"]).astype(np.float32))
    colraw = np.max(np.stack(colps), axis=(0, 1)).astype(np.float64)  # [N]
    colmax = np.empty(N, dtype=np.float64)
    colmax[: N - PLW] = mhat + np.log(np.maximum(colraw[: N - PLW], 1e-300)) / beta
    colmax[N - PLW :] = colraw[N - PLW :]  # plain-C stripe
    return rowmax, colmax, res


def _entropy(m):
    # -sum(exp(c)*c), c = logprob_Normal(1,SIGMA)(1 - m); accumulate in f64
    z = -m.astype(np.float64) / SIGMA
    c = -0.5 * z * z - np.log(SIGMA) - 0.5 * np.log(2.0 * np.pi)
    return -np.sum(np.exp(c) * c)


def kernel(ex, ey):
    ex = np.ascontiguousarray(np.asarray(ex), dtype=np.float32)
    ey = np.ascontiguousarray(np.asarray(ey), dtype=np.float32)
    rowmax, colmax, _ = run_device(ex, ey)
    out1 = np.float32(_entropy(rowmax))
    out2 = np.float32(_entropy(colmax))
    return (np.asarray(out1, dtype=np.float32), np.asarray(out2, dtype=np.float32))


# revision 34
# speedup vs baseline: 1.0538x; 1.0469x over previous
"""Trainium2 Bass kernel for nn_DKL_45810121179236 (retrieval_knn).

Reference computation:
    C = cos_sim_matrix(ex, ey)            # [8192, 8192], D=256
    out1 = -sum(exp(c1)*c1), c1 = logN(1 - rowmax(C))
    out2 = -sum(exp(c2)*c2), c2 = logN(1 - colmax(C))

Strategy: the ACT drain of each PSUM group applies exp(beta*(C - mhat))
(one pass it must make anyway), which gives BOTH outputs nearly free:
  rowmax_i = mhat + ln(sum_j exp(beta*(C_ij-mhat)))/beta - debias
      via the ACTIVATE accum_out per-row sum (logsumexp == max for large
      beta; the EVT first-order bias ln(1+1/(beta*theta))/beta is
      subtracted on host),
  colmax_j = mhat + ln(max_i cf_ij)/beta
      via the DVE running tensor_max over the monotone exp-space tiles.
This deletes the per-tile DVE rowmax trees (~40us) from the old design.
beta and -beta*mhat ride in as a [128,2] runtime tensor (per-partition
scale/bias APs), so the NEFF never rebuilds; the host picks beta from a
subsampled estimate of the rowmax/colmax range to keep all exponents in
f32/bf16 range.

Device per core (ex rows sharded 8 ways):
  mm:    per x-tile (128 rows), 4 PSUM groups [128, 2048] f32,
         2 K-chunks x 4 N=512 bf16 matmuls per group (PE)
  drain: ACT Exp(scale*C+bias) PSUM->SBUF bf16 + accum_out row-sum
  colmax: running elementwise max into colacc [128, 8192] (DVE TT 2x bf16)
  out:   rsums [128, 32] f32 (per x-tile per group), colacc bf16 -> host
"""

import sys

sys.path.insert(0, "/opt/trn_rl_repo")

import copy
from contextlib import ExitStack

import numpy as np
import ml_dtypes

import concourse.bass as bass
import concourse.tile as tile
from concourse import mybir
from concourse import bass_utils

# NOTE: walrus --enable-ldw-opt=true was tried to dedup per-matmul
# LDWEIGHTS; it crashes codegen (visitInstLdweights INTERNAL_ERROR), so
# the per-MM weight reloads stay.

N_CORES = 8
N = 8192  # rows in ex and ey
D = 256  # embedding dim
XR = N // N_CORES  # ex rows per core (1024)
NT_X = XR // 128  # 8 x-tiles per core
PSW = 2048  # psum group width (4 banks)
NG = N // PSW  # 4 psum groups per x-tile
PLW = 1408  # plain-C tail stripe width drained by DVE (cols N-PLW..N)
MMW = 512  # matmul moving-operand width (1024 bf16 rejected by walrus codegen)

F32 = mybir.dt.float32
BF16 = mybir.dt.bfloat16
AF = mybir.ActivationFunctionType
ALU = mybir.AluOpType
AX = mybir.AxisListType

SIGMA = 0.3
EPS = 1e-8
ZPOS = 86.0  # exp budget above mhat (bf16 overflow at e^88.7)
ZNEG = 88.0  # exp budget below mhat (f32/bf16 underflow ~ e^-87)
BETA_MAX = 1500.0


def _split_multi_waits(nc, max_waits=1):
    """The walrus build in this container rejects instructions carrying more
    than one sync wait. Move excess waits onto preceding same-engine NOPs
    (waits on one engine are sequential, so semantics are unchanged)."""
    n_split = 0
    for function in nc.m.functions:
        new_blocks = []
        for block in function.blocks:
            new_insts = []
            for inst in block.instructions:
                si = inst.sync_info
                if si is not None and si.on_wait and len(si.on_wait) > max_waits:
                    waits = list(si.on_wait)
                    n_split += 1
                    head, rest = waits[:-max_waits], waits[-max_waits:]
                    for ci in range(0, len(head), max_waits):
                        new_insts.append(
                            mybir.InstNoOp(
                                name=f"{inst.name}-ws{ci}",
                                engine=inst.engine,
                                sync_info=mybir.SyncInfo(
                                    on_wait=head[ci : ci + max_waits], on_update=[]
                                ),
                            )
                        )
                    inst = copy.replace(
                        inst,
                        sync_info=mybir.SyncInfo(
                            on_wait=rest, on_update=list(si.on_update)
                        ),
                    )
                new_insts.append(inst)
            new_blocks.append(copy.replace(block, instructions=new_insts))
        function.blocks.clear()
        for b in new_blocks:
            function.blocks.append(b)
    return n_split


def _build():
    nc = bass.Bass("TRN2", target_bir_lowering=False, debug=False, num_devices=1)
    # host packs x as [p, h*XR+c] = xn.T[h*128+p, c] (both K-halves along
    # free axis, contiguous 4KB rows); y as [2g+h, p, c] chunk-major blocks
    xT = nc.dram_tensor("xT", [128, 2 * XR], BF16, kind="ExternalInput").ap()
    yT = nc.dram_tensor("yT", [2 * NG, 128, PSW], BF16, kind="ExternalInput").ap()
    sb = nc.dram_tensor("sb", [128, 2], F32, kind="ExternalInput").ap()
    rs_o = nc.dram_tensor("rsums", [128, NT_X * NG], F32, kind="ExternalOutput").ap()
    colp_o = nc.dram_tensor("colp", [128, N], BF16, kind="ExternalOutput").ap()

    with tile.TileContext(nc) as tc:
        with ExitStack() as ctx:
            ep = ctx.enter_context

            persist = ep(tc.tile_pool(name="persist", bufs=1))
            xTab = persist.tile([128, 2 * XR], BF16, tag="xTab")
            yTa = persist.tile([128, N], BF16, tag="yTa")
            yTb = persist.tile([128, N], BF16, tag="yTb")
            colacc = persist.tile([128, N], BF16, tag="colacc")
            sbt = persist.tile([128, 2], F32, tag="sbt")
            rsums = persist.tile([128, NT_X * NG], F32, tag="rsums")


            # input DMAs spread over 4 engine HWDGE queues (contiguous
            # sources); ordered so mm group 0's tiles land first
            def ya(g):
                return (yTa[:, g * PSW : (g + 1) * PSW], yT[2 * g])

            def yb(g):
                return (yTb[:, g * PSW : (g + 1) * PSW], yT[2 * g + 1])

            # first chunks split in half for a shorter pipeline head
            nc.sync.dma_start(yTa[:, 0:1024], yT[0, :, 0:1024])
            nc.sync.dma_start(yTa[:, 1024:2048], yT[0, :, 1024:2048])
            nc.sync.dma_start(*ya(1))
            nc.sync.dma_start(*ya(3))
            nc.scalar.dma_start(yTb[:, 0:1024], yT[1, :, 0:1024])
            nc.scalar.dma_start(yTb[:, 1024:2048], yT[1, :, 1024:2048])
            nc.scalar.dma_start(*yb(1))
            nc.scalar.dma_start(*yb(3))
            nc.gpsimd.dma_start(xTab[:, 0:XR], xT[:, 0:XR])
            nc.gpsimd.dma_start(xTab[:, XR : 2 * XR], xT[:, XR : 2 * XR])
            nc.gpsimd.dma_start(sbt[:], sb)
            nc.gpsimd.dma_start(*ya(2))
            nc.gpsimd.dma_start(*yb(2))

            mm_pool = ep(tc.tile_pool(name="mm", bufs=2, space="PSUM"))
            cf_pool = ep(tc.tile_pool(name="cf", bufs=2))

            for xt in range(NT_X):
                xa = slice(xt * 128, (xt + 1) * 128)
                xb = slice(XR + xt * 128, XR + (xt + 1) * 128)
                cf = cf_pool.tile([128, N], BF16, tag="cf")
                for g in range(NG):
                    j0 = g * PSW
                    ps = mm_pool.tile([128, PSW], F32, tag="mm")
                    for c in range(PSW // MMW):
                        nc.tensor.matmul(
                            ps[:, c * MMW : (c + 1) * MMW],
                            xTab[:, xa],
                            yTa[:, j0 + c * MMW : j0 + (c + 1) * MMW],
                            start=True,
                            stop=False,
                        )
                    for c in range(PSW // MMW):
                        nc.tensor.matmul(
                            ps[:, c * MMW : (c + 1) * MMW],
                            xTab[:, xb],
                            yTb[:, j0 + c * MMW : j0 + (c + 1) * MMW],
                            start=False,
                            stop=True,
                        )
                    # fused drain: cf = exp(beta*C - beta*mhat) bf16,
                    # rsums[:, xt*NG+g] = sum_j cf (f32)
                    nc.scalar.activation(
                        cf[:, j0 : j0 + PSW],
                        ps[:],
                        AF.Exp,
                        bias=sbt[:, 1:2],
                        scale=sbt[:, 0:1],
                        accum_out=rsums[:, xt * NG + g : xt * NG + g + 1],
                    )

                    # colmax chunk for (xt, g) right after its drain (walrus
                    # rejects TensorTensor on GpSimd, so DVE owns all of it)
                    if xt == 0:
                        nc.vector.tensor_copy(
                            colacc[:, j0 : j0 + PSW], cf[:, j0 : j0 + PSW]
                        )
                    else:
                        nc.vector.tensor_max(
                            colacc[:, j0 : j0 + PSW],
                            colacc[:, j0 : j0 + PSW],
                            cf[:, j0 : j0 + PSW],
                        )
                        if xt == NT_X - 1:
                            nc.sync.dma_start(
                                colp_o[:, j0 : j0 + PSW], colacc[:, j0 : j0 + PSW]
                            )

            nc.scalar.dma_start(rs_o, rsums[:])

    _split_multi_waits(nc)
    return nc


_NC_CACHE = []


def _get_nc():
    if not _NC_CACHE:
        _NC_CACHE.append(_build())
    return _NC_CACHE[0]


def run_device(ex, ey, trace=False):
    """Normalize/transposed-shard on host, run SPMD kernel, return
    (rowmax [N], colmax [N], results obj)."""
    nc = _get_nc()
    xn = ex / np.maximum(np.linalg.norm(ex, axis=-1, keepdims=True), EPS)
    yn = ey / np.maximum(np.linalg.norm(ey, axis=-1, keepdims=True), EPS)
    xTf = np.ascontiguousarray(xn.T).astype(ml_dtypes.bfloat16)  # [256, 8192]
    yTf = np.ascontiguousarray(yn.T).astype(ml_dtypes.bfloat16)
    # chunk-major contiguous device layouts (see _build)
    yTq = np.ascontiguousarray(
        yTf.reshape(2, 128, NG, PSW).transpose(2, 0, 1, 3).reshape(2 * NG, 128, PSW)
    )

    # host-side beta/mhat selection from a subsampled range estimate
    S = 4
    rm_est = (xn @ yn[::S].T).max(axis=1)
    cm_est = (xn[::S] @ yn.T).max(axis=0)
    sigma_c = float(np.std((xn[::97] @ yn[::89].T)))
    z_full = np.sqrt(2.0 * np.log(float(N)))
    z_sub = np.sqrt(2.0 * np.log(float(N) / S))
    delta = sigma_c * (z_full - z_sub)
    hi = float(max(rm_est.max(), cm_est.max())) + 0.65 * delta + 0.004
    lo = float(min(rm_est.min(), cm_est.min())) + 0.55 * delta - 0.004
    hi = min(hi, 1.0)  # cosine bound
    lo = min(lo, hi - 1e-3)
    beta = min(BETA_MAX, (ZPOS + ZNEG) / (hi - lo))
    mhat = hi - ZPOS / beta
    theta = sigma_c / z_full
    debias = float(np.log1p(1.0 / (beta * theta)) / beta)

    sbv = np.empty((128, 2), dtype=np.float32)
    sbv[:, 0] = beta
    sbv[:, 1] = -beta * mhat

    in_maps = [
        {
            "xT": np.ascontiguousarray(
                xTf[:, k * XR : (k + 1) * XR]
                .reshape(2, 128, XR)
                .transpose(1, 0, 2)
                .reshape(128, 2 * XR)
            ),
            "yT": yTq,
            "sb": sbv,
        }
        for k in range(N_CORES)
    ]
    res = bass_utils.run_bass_kernel_spmd(
        nc, in_maps, core_ids=list(range(N_CORES)), trace=trace
    )
    rowmax = np.empty(N, dtype=np.float64)
    colps = []
    for k in range(N_CORES):
        rs = np.asarray(res.results[k]["rsums"], dtype=np.float64)  # [128, 32]
        # row xt*128+p of this shard -> rsums[p, xt*NG:(xt+1)*NG]
        rs = rs.reshape(128, NT_X, NG).sum(axis=2)  # [128, NT_X]
        rowsum = np.maximum(rs.T.reshape(-1), 1e-300)  # [XR], row-major
        rowmax[k * XR : (k + 1) * XR] = mhat + np.log(rowsum) / beta - debias
        colps.append(np.asarray(res.results[k]["colp"]).astype(np.float32))
    colraw = np.max(np.stack(colps), axis=(0, 1)).astype(np.float64)  # [N]
    colmax = mhat + np.log(np.maximum(colraw, 1e-300)) / beta
    return rowmax, colmax, res


def _entropy(m):
    # -sum(exp(c)*c), c = logprob_Normal(1,SIGMA)(1 - m); accumulate in f64
    z = -m.astype(np.float64) / SIGMA
    c = -0.5 * z * z - np.log(SIGMA) - 0.5 * np.log(2.0 * np.pi)
    return -np.sum(np.exp(c) * c)


def kernel(ex, ey):
    ex = np.ascontiguousarray(np.asarray(ex), dtype=np.float32)
    ey = np.ascontiguousarray(np.asarray(ey), dtype=np.float32)
    rowmax, colmax, _ = run_device(ex, ey)
    out1 = np.float32(_entropy(rowmax))
    out2 = np.float32(_entropy(colmax))
    return (np.asarray(out1, dtype=np.float32), np.asarray(out2, dtype=np.float32))
